# revision 19
# baseline (speedup 1.0000x reference)
"""Gaussian RBF network kernel for 8 Trainium2 NeuronCores.

Computes out[n] = sum_c w[c] * exp(-0.5 * (x_n - c_c)^T P (x_n - c_c)),
P = L @ L.T from packed lower-triangular elements, N=8192, C=512, F=128.

Strategy: data-parallel over N (1024 rows per core).  With G = L.T the
exponent is -0.5*||G x - G c||^2, so the host precomputes the factor
matrices Gx = G @ X.T and Gc = G @ C.T in fp8e4 (the norms qx/qc are
taken of the *rounded* factors, so the exponent stays an exact negative
quadratic form plus ln|w| and can never overflow), and the device does
only the O(N*C*F) work.

Per 128-row tile t (layout [n_partition, c_free], centers sorted w>0
first, qcw[c] = -0.5*qc[c] + ln|w_c|):
  A[n, c]  = Gx[:, n] . Gc[:, c]           (PE, fp8e4 in, f32 PSUM)
  A[n, c] += ones.T @ (qcw/128) = qcw[c]   (PE fold, same-dtype group)
  phi      = exp(A + qx[n])                (Scalar, PSUM in, bias AP)
  acc_p/n[t] = sum_c phi over w>0 / w<=0   (DVE tensor_reduce pairs for
              t<7; the last tile reduces inside the activation via
              accum_out so the stream ends with the last exp)
out[n] = acc_p[n] - acc_n[n], PE-transposed to [8, 128] and DMA'd out.
"""

import contextlib
import ctypes
import sys
import types

import numpy as np

N, C, F = 8192, 512, 128
NCORES = 8
NC = N // NCORES  # rows per core
NT = NC // 128    # 128-row n-tiles per core
N_ACC = 1         # trailing tiles reduced on the Scalar engine (accum_out)

_cache = {}


def _install_ntff_hook():
    """bass_utils wants antenv.axon_hooks for trace=True under axon; the
    image lacks it. Provide the same ctypes hook trn_boot would install.
    Degrades silently if anything is off (tracing just gets skipped)."""
    if "antenv.axon_hooks" in sys.modules:
        return
    try:
        import antenv

        so_path = "/opt/axon/libaxon_pjrt.so"
        lib = ctypes.CDLL(so_path)
        if not hasattr(lib, "axon_start_nrt_profile"):
            return
        lib.axon_start_nrt_profile.argtypes = [
            ctypes.POINTER(ctypes.c_int64),
            ctypes.c_size_t,
        ]
        lib.axon_start_nrt_profile.restype = ctypes.c_int64
        lib.axon_stop_nrt_profile.argtypes = [ctypes.c_char_p]
        lib.axon_stop_nrt_profile.restype = ctypes.c_int64

        @contextlib.contextmanager
        def _hook(output_dir, device_ids):
            import jax
            import numpy as _np

            # Profiling start fails (rc=-1) until the axon terminal has
            # dispatched at least one computation; warm it with a tiny op.
            d0 = jax.devices()[0]
            x = jax.device_put(_np.ones((2, 2), _np.float32), d0)
            (x + x).block_until_ready()
            if device_ids:
                ids = (ctypes.c_int64 * len(device_ids))(*device_ids)
                rc = lib.axon_start_nrt_profile(ids, len(device_ids))
            else:
                rc = lib.axon_start_nrt_profile(None, 0)
            try:
                yield
            finally:
                if rc == 0:
                    lib.axon_stop_nrt_profile(str(output_dir).encode())

        mod = types.ModuleType("antenv.axon_hooks")
        mod.get_axon_ntff_profile_hook = lambda: _hook
        mod.set_axon_ntff_profile_hook = lambda h: None
        sys.modules["antenv.axon_hooks"] = mod
        antenv.axon_hooks = mod
    except Exception:
        pass


def _build(npos):
    import concourse.bass as bass
    import concourse.mybir as mybir
    import concourse.tile as tile
    from concourse import bacc

    f32 = mybir.dt.float32
    bf16 = mybir.dt.bfloat16
    f8e4 = mybir.dt.float8e4
    Exp = mybir.ActivationFunctionType.Exp
    Alu = mybir.AluOpType
    X_ax = mybir.AxisListType.X

    nc = bacc.Bacc(
        "TRN2", target_bir_lowering=False, debug=False, num_devices=NCORES
    )
    # gc data in cols 0:512, all-ones lhsT block in cols 512:640
    gc_d = nc.dram_tensor("gc", [F, C + 128], f8e4, kind="ExternalInput")
    gx0_d = nc.dram_tensor("gx0", [F, 512], f8e4, kind="ExternalInput")
    gx1_d = nc.dram_tensor("gx1", [F, 512], f8e4, kind="ExternalInput")
    qcw_d = nc.dram_tensor("qcw", [F, C], f8e4, kind="ExternalInput")
    aux_d = nc.dram_tensor("aux", [F, 16], bf16, kind="ExternalInput")
    # raw acc_p | acc_n; the subtract + transpose happen on the host
    out_d = nc.dram_tensor("out", [F, 2 * NT], f32, kind="ExternalOutput")

    with tile.TileContext(nc) as tc:
        with (
            tc.tile_pool(name="sb", bufs=1) as sb,
            tc.tile_pool(name="phip", bufs=4) as phip,
            tc.tile_pool(name="mm", bufs=4, space=bass.MemorySpace.PSUM) as mm,
            tc.tile_pool(name="warm", bufs=1, space=bass.MemorySpace.PSUM) as warmp,
        ):
            # ---- loads: gc + gx1 on the SP HWDGE queue, gx0 + qcw on the
            # Act HWDGE queue (so gc and gx0 land in parallel), aux on the
            # gpsimd SWDGE ----
            gc_sb = sb.tile([F, C + 128], f8e4)
            nc.sync.dma_start(gc_sb[:], gc_d[:])
            gx0_sb = sb.tile([F, 512], f8e4)
            nc.scalar.dma_start(gx0_sb[:], gx0_d[:])
            qcw_sb = sb.tile([F, C], f8e4)
            nc.sync.dma_start(qcw_sb[:], qcw_d[:])
            gx1_sb = sb.tile([F, 512], f8e4)
            nc.sync.dma_start(gx1_sb[:], gx1_d[:])
            aux_sb = sb.tile([F, 16], bf16)
            nc.gpsimd.dma_start(aux_sb[:], aux_d[:])
            qx_ap = aux_sb[:, 0:16].bitcast(f32)  # [F, NT] f32
            qcw_div = qcw_sb[:, 0:C]
            ones_blk = gc_sb[:, C : C + 128]

            acc = sb.tile([F, 2 * NT], f32, tag="acc")
            accp = acc[:, 0:NT]
            accn = acc[:, NT : 2 * NT]

            # ---- PE warmup on the first-arriving tensor (gc): flip the
            # HAM clock gate to full rate AND warm the 512-col matmul
            # config so the first qcw-fold doesn't pay the slow path ----
            warm_ps = warmp.tile([F, 512], f32, tag="warm")
            nc.tensor.matmul(
                warm_ps[:, 0:128], gc_sb[:, 0:128], gc_sb[:, 0:128],
                start=True, stop=True,
            )
            nc.tensor.matmul(
                warm_ps[:], ones_blk, gc_sb[:, 0:C],
                start=True, stop=True,
            )

            for t in range(NT):
                gx_t = gx0_sb if t < 4 else gx1_sb
                lo = (t % 4) * 128
                a_ps = mm.tile([128, C], f32, tag="mm")
                nc.tensor.matmul(
                    a_ps[:], gx_t[:, lo : lo + 128], gc_sb[:, 0:C],
                    start=True, stop=False,
                )
                # fold qcw into PSUM: ones.T @ (qcw/128) adds qcw[c] to
                # every row; same-dtype K=128 group as the A matmul
                nc.tensor.matmul(
                    a_ps[:], ones_blk, qcw_div,
                    start=False, stop=True,
                )
                phi = phip.tile([128, C], bf16, tag="phi")
                qx_t = qx_ap[:, t : t + 1]
                if t >= NT - N_ACC:
                    # scalar-side reduction: pos/neg split with accum_out;
                    # the reduction rides inside the exp, no post-exp tail
                    if npos > 0:
                        nc.scalar.activation(
                            phi[:, 0:npos], a_ps[:, 0:npos], Exp,
                            bias=qx_t, accum_out=accp[:, t : t + 1],
                        )
                    else:
                        nc.vector.memset(accp[:, t : t + 1], 0.0)
                    if npos < C:
                        nc.scalar.activation(
                            phi[:, npos:C], a_ps[:, npos:C], Exp,
                            bias=qx_t, accum_out=accn[:, t : t + 1],
                        )
                    else:
                        nc.vector.memset(accn[:, t : t + 1], 0.0)
                else:
                    # full-width exp from PSUM, pos/neg reductions on DVE
                    nc.scalar.activation(phi[:], a_ps[:], Exp, bias=qx_t)
                    if npos > 0:
                        nc.vector.tensor_reduce(
                            accp[:, t : t + 1], phi[:, 0:npos],
                            axis=X_ax, op=Alu.add,
                        )
                    else:
                        nc.vector.memset(accp[:, t : t + 1], 0.0)
                    if npos < C:
                        nc.vector.tensor_reduce(
                            accn[:, t : t + 1], phi[:, npos:C],
                            axis=X_ax, op=Alu.add,
                        )
                    else:
                        nc.vector.memset(accn[:, t : t + 1], 0.0)

            nc.sync.dma_start(out_d[:], acc[:])

    nc.compile()
    return nc


def _prep_inputs(X, precision_elements, centers, weights):
    import ml_dtypes

    bf = ml_dtypes.bfloat16
    f8e4 = ml_dtypes.float8_e4m3

    ti, tj = np.tril_indices(F)
    L = np.zeros((F, F), np.float32)
    L[ti, tj] = precision_elements
    G = L.T  # exponent = -0.5 ||G x - G c||^2

    Gx8 = (G @ X.astype(np.float32).T).astype(f8e4)  # [F, N]
    Gxr = Gx8.astype(np.float32)
    qx = -0.5 * (Gxr * Gxr).sum(0)  # [N] f32, of the *rounded* factors

    pos = weights > 0
    npos = int(pos.sum())
    perm = np.concatenate([np.nonzero(pos)[0], np.nonzero(~pos)[0]])
    Gc8 = np.ascontiguousarray((G @ centers.astype(np.float32).T)[:, perm]).astype(f8e4)
    Gcr = Gc8.astype(np.float32)
    qc = (Gcr * Gcr).sum(0)  # [C]
    with np.errstate(divide="ignore"):
        lnw = np.log(np.abs(weights[perm].astype(np.float64))).astype(np.float32)
    qcw_row = -0.5 * qc + lnw
    qcw_t = np.ascontiguousarray(
        np.broadcast_to(qcw_row / 128.0, (F, C))
    ).astype(f8e4)
    gc_full = np.ones((F, C + 128), f8e4)
    gc_full[:, 0:C] = Gc8
    in_maps = []
    for s in range(NCORES):
        qx_c = np.ascontiguousarray(
            qx[s * NC : (s + 1) * NC].reshape(NT, 128).T
        )  # [128, NT] f32: column t holds qx for n = t*128 + p
        aux = qx_c.astype("<f4").view("<u2").reshape(F, 2 * NT)
        in_maps.append(
            {
                "gc": gc_full,
                "gx0": np.ascontiguousarray(Gx8[:, s * NC : s * NC + 512]),
                "gx1": np.ascontiguousarray(Gx8[:, s * NC + 512 : (s + 1) * NC]),
                "qcw": qcw_t,
                "aux": aux.view(bf),
            }
        )
    return in_maps, npos


def kernel(X, precision_elements, centers, weights):
    _install_ntff_hook()
    from concourse.bass_utils import run_bass_kernel_spmd

    in_maps, npos = _prep_inputs(X, precision_elements, centers, weights)
    key = ("nc", npos)
    if key not in _cache:
        _cache[key] = _build(npos)
    nc = _cache[key]

    res = run_bass_kernel_spmd(nc, in_maps, core_ids=list(range(NCORES)))
    _cache["last_results"] = res
    outs = []
    for r in res.results:
        acc = np.asarray(r["out"], np.float32)  # [128, 2*NT]: acc_p | acc_n
        outs.append((acc[:, 0:NT] - acc[:, NT:]).T.reshape(NC))
    return np.concatenate(outs).astype(np.float32)
